# revision 4
# baseline (speedup 1.0000x reference)
"""Trainium2 Bass kernel for privacy-aware token pruning (topk + gather + masked mean).

Reference semantics (per batch row b, N=4096 tokens, D=1024, k=2048):
  top_idx = top_k(attn[b], k)                  # sorted by value desc, ties by index asc
  out[b, 0:k]  = seq[b, top_idx]               # gathered in sorted order
  out[b, k]    = 0.05 * sum(seq[b, pruned]) / (N - k)

Device algorithm (pure data parallel, 2 batch rows per core, 8 cores):
  For each row:
    1. rank_i = #{j<i: v_j >= v_i} + #{j>i: v_j > v_i}   (exact top_k order incl.
       index tie-break). Computed on DVE with fused compare+accumulate
       (tensor_scalar/scalar_tensor_tensor with accum_out), 128 tokens per
       partition-chunk, j streamed along the free axis.
    2. pruned mask = rank >= k; masked token sum via PE matmul (mask as lhsT),
       accumulated over token chunks in PSUM; scaled on ACT into the mixup token.
    3. Tokens streamed through SBUF and scattered straight to their output slot
       with indirect DMA (offset = rank + row_base); pruned tokens fall outside
       the per-call bounds_check and are silently dropped (no trash writes).
"""

import numpy as np

import concourse.bass as bass
import concourse.tile as tile
from concourse import bacc, mybir
from concourse.bass import IndirectOffsetOnAxis
from concourse.bass_utils import run_bass_kernel_spmd

F32 = mybir.dt.float32
BF16 = mybir.dt.bfloat16
I32 = mybir.dt.int32

B_FULL, N_FULL, D_FULL = 16, 4096, 1024
N_CORES = 8
K_FULL = N_FULL // 2


def build_program(b_per_core=2, n=N_FULL, d=D_FULL, k=K_FULL, loop=False, n_cores=N_CORES,
                  internal_seq=False):
    """Build the per-core SPMD Bass program. Returns the compiled Bacc object.

    internal_seq=True replaces the seq input with an uninitialized internal DRAM
    tensor — timing-only variant that avoids shipping 32 MiB/core per run
    (scatter pattern / instruction stream are unchanged; token values are garbage).
    """
    ch = n // 128           # token chunks of 128
    dh = d // 2             # matmul N slice (<=512)
    assert dh <= 512 and ch % 2 == 0
    # rem_cnt in the reference is sum(mask) + 1e-10 evaluated in f32 == float(n-k)
    scale = float(np.float32(0.05) / np.float32(np.float32(n - k) + 1e-10))

    nc = bacc.Bacc("TRN2", target_bir_lowering=False, debug=False, num_devices=n_cores)
    if internal_seq:
        seq = nc.dram_tensor("seq_internal", [b_per_core, n, d], F32).ap()
    else:
        seq = nc.dram_tensor("seq", [b_per_core, n, d], F32, kind="ExternalInput").ap()
    attn = nc.dram_tensor("attn", [b_per_core, n], F32, kind="ExternalInput").ap()
    # attn_t[b, p, c] = attn[b, c*128 + p] (host-transposed layout for the
    # per-partition compare scalars)
    attn_t = nc.dram_tensor("attn_t", [b_per_core, 128, ch], F32, kind="ExternalInput").ap()
    # ltgt[:, 0:128] = LT (1.0 where f < p), ltgt[:, 128:256] = GT (1.0 where f > p)
    ltgt = nc.dram_tensor("ltgt", [128, 256], F32, kind="ExternalInput").ap()
    if loop:
        reps = nc.dram_tensor("reps", [1, 1], I32, kind="ExternalInput").ap()
    out_tok = nc.dram_tensor("out_tok", [b_per_core * k, d], F32, kind="ExternalOutput").ap()
    out_rem = nc.dram_tensor("out_rem", [b_per_core, d], F32, kind="ExternalOutput").ap()

    with tile.TileContext(nc) as tc:
        with (
            tc.tile_pool(name="const", bufs=1) as constp,
            tc.tile_pool(name="xb", bufs=2) as xbp,
            tc.tile_pool(name="dummy", bufs=2) as dummyp,
            tc.tile_pool(name="pairs", bufs=6) as pairp,
            tc.tile_pool(name="small", bufs=2) as smallp,
            tc.tile_pool(name="psum", bufs=2, space="PSUM") as psump,
        ):
            ltgt_sb = constp.tile([128, 256], F32)
            nc.sync.dma_start(ltgt_sb[:], ltgt[:])
            lt_m = ltgt_sb[:, 0:128]
            gt_m = ltgt_sb[:, 128:256]

            def body():
                for b in range(b_per_core):
                    # ---- per-row attn tiles ----
                    xb = xbp.tile([128, n], F32, tag="xb")
                    nc.sync.dma_start(xb[:], attn[b : b + 1, :].to_broadcast([128, n]))
                    vch = smallp.tile([128, ch], F32, tag="vch")
                    nc.sync.dma_start(vch[:], attn_t[b])

                    # ---- ranks (exact, with index tie-break) ----
                    pfx = smallp.tile([128, ch], F32, tag="pfx")
                    sfx = smallp.tile([128, ch], F32, tag="sfx")
                    owna = smallp.tile([128, ch], F32, tag="owna")
                    ownb = smallp.tile([128, ch], F32, tag="ownb")
                    dummy_big = dummyp.tile([128, n], BF16, tag="dummy_big")
                    dummy_own = dummyp.tile([128, 128], BF16, tag="dummy_own")
                    for c in range(ch):
                        scal = vch[:, c : c + 1]
                        lo, hi = c * 128, (c + 1) * 128
                        if c > 0:
                            nc.vector.tensor_scalar(
                                out=dummy_big[:, :lo],
                                in0=xb[:, :lo],
                                scalar1=scal,
                                scalar2=None,
                                op0=mybir.AluOpType.is_ge,
                                op1=mybir.AluOpType.add,
                                accum_out=pfx[:, c : c + 1],
                            )
                        else:
                            nc.vector.memset(pfx[:, 0:1], 0.0)
                        if c < ch - 1:
                            nc.vector.tensor_scalar(
                                out=dummy_big[:, : n - hi],
                                in0=xb[:, hi:],
                                scalar1=scal,
                                scalar2=None,
                                op0=mybir.AluOpType.is_gt,
                                op1=mybir.AluOpType.add,
                                accum_out=sfx[:, c : c + 1],
                            )
                        else:
                            nc.vector.memset(sfx[:, c : c + 1], 0.0)
                        nc.vector.scalar_tensor_tensor(
                            out=dummy_own[:],
                            in0=xb[:, lo:hi],
                            scalar=scal,
                            in1=lt_m,
                            op0=mybir.AluOpType.is_ge,
                            op1=mybir.AluOpType.mult,
                            accum_out=owna[:, c : c + 1],
                        )
                        nc.vector.scalar_tensor_tensor(
                            out=dummy_own[:],
                            in0=xb[:, lo:hi],
                            scalar=scal,
                            in1=gt_m,
                            op0=mybir.AluOpType.is_gt,
                            op1=mybir.AluOpType.mult,
                            accum_out=ownb[:, c : c + 1],
                        )
                    rank = smallp.tile([128, ch], F32, tag="rank")
                    nc.vector.tensor_tensor(
                        out=rank[:], in0=pfx[:], in1=sfx[:], op=mybir.AluOpType.add
                    )
                    nc.vector.tensor_tensor(
                        out=owna[:], in0=owna[:], in1=ownb[:], op=mybir.AluOpType.add
                    )
                    nc.vector.tensor_tensor(
                        out=rank[:], in0=rank[:], in1=owna[:], op=mybir.AluOpType.add
                    )

                    # scatter offsets (row base folded in) and pruned mask
                    offs = smallp.tile([128, ch], I32, tag="offs")
                    nc.vector.tensor_scalar(
                        out=offs[:],
                        in0=rank[:],
                        scalar1=float(b * k),
                        scalar2=None,
                        op0=mybir.AluOpType.add,
                    )
                    maskp = smallp.tile([128, ch], F32, tag="maskp")
                    nc.vector.tensor_scalar(
                        out=maskp[:],
                        in0=rank[:],
                        scalar1=float(k) - 0.5,
                        scalar2=None,
                        op0=mybir.AluOpType.is_gt,
                    )

                    # ---- stream tokens: masked-sum matmul + indirect scatter ----
                    ps_a = psump.tile([1, dh], F32, tag="psa")
                    ps_b = psump.tile([1, dh], F32, tag="psb")
                    for t in range(ch // 2):
                        pair = pairp.tile([128, 2 * d], F32, tag="pair")
                        src = seq[b, t * 256 : (t + 1) * 256, :].rearrange(
                            "(two p) d -> p two d", p=128
                        )
                        dst = pair[:].rearrange("p (two d) -> p two d", two=2)
                        nc.sync.dma_start(dst, src)
                        for h in (0, 1):
                            c = 2 * t + h
                            sub = pair[:, h * d : (h + 1) * d]
                            nc.tensor.matmul(
                                out=ps_a[:],
                                lhsT=maskp[:, c : c + 1],
                                rhs=sub[:, 0:dh],
                                start=(c == 0),
                                stop=(c == ch - 1),
                            )
                            nc.tensor.matmul(
                                out=ps_b[:],
                                lhsT=maskp[:, c : c + 1],
                                rhs=sub[:, dh:d],
                                start=(c == 0),
                                stop=(c == ch - 1),
                            )
                            nc.gpsimd.indirect_dma_start(
                                out=out_tok,
                                out_offset=IndirectOffsetOnAxis(
                                    ap=offs[:, c : c + 1], axis=0
                                ),
                                in_=sub,
                                in_offset=None,
                                bounds_check=b * k + (k - 1),
                                oob_is_err=False,
                            )

                    # ---- mixup token ----
                    rem_sb = smallp.tile([1, d], F32, tag="rem")
                    nc.scalar.mul(rem_sb[:, 0:dh], ps_a[:], scale)
                    nc.scalar.mul(rem_sb[:, dh:d], ps_b[:], scale)
                    nc.sync.dma_start(out_rem[b : b + 1, :], rem_sb[:])

            if loop:
                repst = constp.tile([1, 1], I32)
                nc.sync.dma_start(repst[:], reps[:])
                rv = nc.values_load(repst[0:1, 0:1], skip_runtime_bounds_check=True)
                with tc.For_i(0, rv, 1):
                    body()
            else:
                body()

    nc.compile()
    return nc


def make_const_inputs():
    p = np.arange(128)
    lt = (p[None, :] < p[:, None]).astype(np.float32)
    gt = (p[None, :] > p[:, None]).astype(np.float32)
    return np.concatenate([lt, gt], axis=1)  # [128, 256]


def make_in_maps(seq, attn, n_cores=N_CORES, reps=None):
    ltgt = make_const_inputs()
    b_per_core = seq.shape[0] // n_cores
    ch = seq.shape[1] // 128
    in_maps = []
    for c in range(n_cores):
        rows = slice(c * b_per_core, (c + 1) * b_per_core)
        a = np.ascontiguousarray(attn[rows]).astype(np.float32, copy=False)
        a_t = np.ascontiguousarray(
            a.reshape(b_per_core, ch, 128).transpose(0, 2, 1)
        )
        m = {
            "seq": np.ascontiguousarray(seq[rows]).astype(np.float32, copy=False),
            "attn": a,
            "attn_t": a_t,
            "ltgt": ltgt,
        }
        if reps is not None:
            m["reps"] = np.array([[reps]], dtype=np.int32)
        in_maps.append(m)
    return in_maps


def assemble_output(results, b_per_core, n_cores, k, d):
    out = np.empty((b_per_core * n_cores, k + 1, d), dtype=np.float32)
    for c in range(n_cores):
        tok = results[c]["out_tok"].reshape(b_per_core, k, d)
        rem = results[c]["out_rem"]
        rows = slice(c * b_per_core, (c + 1) * b_per_core)
        out[rows, :k] = tok
        out[rows, k] = rem
    return out


_PROGRAM_CACHE = {}


def _get_program():
    if "nc" not in _PROGRAM_CACHE:
        _PROGRAM_CACHE["nc"] = build_program()
    return _PROGRAM_CACHE["nc"]


def kernel(seq, attn_weights):
    seq = np.asarray(seq, dtype=np.float32)
    attn = np.asarray(attn_weights, dtype=np.float32)
    nc = _get_program()
    in_maps = make_in_maps(seq, attn)
    res = run_bass_kernel_spmd(nc, in_maps, list(range(N_CORES)))
    return assemble_output(res.results, B_FULL // N_CORES, N_CORES, K_FULL, D_FULL)
